# revision 1
# baseline (speedup 1.0000x reference)
"""Trainium2 Bass kernel for AdaptiveMixtureOfExperts (top-2 SwiGLU MoE).

Strategy (expert-parallel with FF-split load balancing):
  - Host computes the tiny router (x @ Wr, top-2, softmax) with jax-on-CPU ops
    that bit-match the reference, then shards tokens by routed expert.
  - Each expert's FFN is split in half along D_FF; each NeuronCore runs two
    half-FFN "sections": section A = FF-half h of one of the 4 largest
    experts, section B = FF-half h of one of the 4 smallest (cores 0-3 take
    h=0, cores 4-7 take h=1).  This balances per-core work to the average of
    a large+small expert instead of the max expert.
        hT = W1h.T @ xgT            (ff on partitions, tokens on free dim)
        uT = (a + b1a) * silu(g + b1g)
        yT_partial = W2h.T @ uT     (+ b2 on host)
  - Host sums the two half contributions per expert, applies the top-2
    combine weights, and scatter-adds into the full [B, S, D] output.

DMA design notes (measured):
  - dma_start issue occupies its HWDGE sequencer ~600ns (128 descriptors)
    regardless of width, so keep chunk count moderate (512-col chunks).
  - [128, 512] per-k transfers (1KB descriptors, ~2.2KB DRAM row pitch)
    sustain ~380GB/s aggregate; k-folded wide-row layouts (4KB descriptors at
    ~17KB pitch) drop to ~200GB/s (HBM page locality) — do NOT fold.
  - 3D-AP rearranged dma_starts take 3-8us of descriptor generation — avoid.

Shapes hardcoded for the problem instance:
  x:[2,2048,1024] f32, Wr:[1024,8], temp:[1], W1:[8,1024,4096], b1:[8,4096],
  W2:[8,2048,1024], b2:[8,1024].  TOP_K=2, 8 experts on 8 cores.
"""

import os

import numpy as np
import ml_dtypes

D_MODEL = 1024
D_FF = 2048
NUM_EXPERTS = 8
TOP_K = 2
P = 128          # partitions
NT = 512         # token tile (moving free dim per matmul)
N_CORES = 8
FH = D_FF // 2   # ff half
NO = D_MODEL // P

_NC_CACHE = {}
LAST_RESULTS = None  # test harness introspection


def _tile_bounds(C):
    n_t = (C + NT - 1) // NT
    cuts = [min(NT * i, C) for i in range(n_t)] + [C]
    return [(cuts[i], cuts[i + 1] - cuts[i]) for i in range(n_t)]


def _tail_split(CA):
    """(has_tail, CA_main, a_last) for section a."""
    n_ta = (CA + NT - 1) // NT
    a_last = CA - NT * (n_ta - 1)
    has_tail = n_ta > 1 and a_last <= NT // 2
    CA_main = NT * (n_ta - 1) if has_tail else CA
    return has_tail, CA_main, a_last


def _build_nc(CA: int, CB: int, use_silu: bool = True):
    """Per-core Bass graph: two half-FF FFN sections of CA and CB tokens.

    use_silu=False decomposes silu into sigmoid+mul (CoreSim has no Silu).
    """
    import concourse.mybir as mybir
    import concourse.tile as tile
    from concourse import bacc
    from concourse.bass import ts

    f32 = mybir.dt.float32
    bf16 = mybir.dt.bfloat16
    AF = mybir.ActivationFunctionType

    K1 = D_MODEL // P          # 8 k-tiles for matmul1
    K2 = FH // P               # 8 k-tiles for matmul2 (half ff)
    NF1 = 2 * FH // P          # 16 ff tiles of hT per section (a+g)
    WCHUNK = 512

    has_tail, CA_main, a_last = _tail_split(CA)
    # plain 512 tiling; the small ragged tail of section a runs at the very
    # end so the exposed drain+DMA tail after the final matmul is minimal.
    # (A small 128-col head tile was tried and regressed: N=128 matmuls can't
    # hide LDWEIGHTS, +9.6us PE busy for ~1us head win.)
    sec_bounds = {"a": _tile_bounds(CA), "b": _tile_bounds(CB)}

    nc = bacc.Bacc()
    xT = {}
    w1 = {}
    w2 = {}
    b1t = {}
    secs = [("a", CA), ("b", CB)]

    for s, C in secs:
        xT[s] = nc.declare_dram_parameter(f"x{s}T", [D_MODEL, C], bf16, isOutput=False)
        w1[s] = nc.declare_dram_parameter(f"w1{s}", [D_MODEL, 2 * FH], bf16, isOutput=False)
        w2[s] = nc.declare_dram_parameter(f"w2{s}", [FH, D_MODEL], bf16, isOutput=False)
        b1t[s] = nc.declare_dram_parameter(f"b1t{s}", [P, NF1], f32, isOutput=False)
    # single merged output tensor (partial y without b2; bf16 halves output
    # DMA bytes): section a columns, then section b, then the packed tail
    # block on rows 0:P only.  One tensor keeps the end-of-kernel completion
    # bookkeeping minimal.
    W_ALL = CA_main + CB + (NO * a_last if has_tail else 0)
    out_all = nc.declare_dram_parameter(
        "outall", [D_MODEL, W_ALL], bf16, isOutput=True)
    out_off = {"a": 0, "b": CA_main}
    # packed tail output, split in two halves so the first half's ~2us HBM
    # write-receipt overlaps the second half's matmuls instead of sitting
    # exposed before the end-of-kernel barrier
    HNO = NO // 2
    out_tail_lo = out_tail_hi = None
    if has_tail:
        W0 = CA_main + CB
        out_tail_lo = out_all[0:P, W0:W0 + HNO * a_last]
        out_tail_hi = out_all[0:P, W0 + HNO * a_last:W_ALL]

    with tile.TileContext(nc) as tc:
        with (
            tc.tile_pool(name="weights", bufs=1) as wpool,
            tc.tile_pool(name="acts", bufs=2) as upool,
            tc.tile_pool(name="epilogue", bufs=4) as epool,
            tc.tile_pool(name="ps", bufs=8, space="PSUM") as ps_pool,
        ):
            # bias tiles: the b1 DMAs are NOT issued first — their 128
            # latency-bound 64B descriptors at the qACT ring head delay the
            # critical x-tile0 transfers ~1.4us (early DMA is latency-bound
            # at ~300ns/descriptor).  b1a rides qACT right after xg-t0
            # (needed at the first activation ~13.8us); b1b rides qSP with
            # section b's inputs (needed ~100us).
            b1_sb = {}
            for s, C in secs:
                b1_sb[s] = wpool.tile([P, NF1], f32, name=f"b1_sb{s}", tag=f"b1{s}")

            xg_sb = {}
            w1_sb = {}
            w2_sb = {}
            for s, C in secs:
                xg_sb[s] = [
                    wpool.tile([P, C], bf16, name=f"xg_sb{s}{k}", tag=f"xg{s}{k}")
                    for k in range(K1)
                ]
                w1_sb[s] = [
                    wpool.tile([P, 2 * FH], bf16, name=f"w1_sb{s}{k}", tag=f"w1{s}{k}")
                    for k in range(K1)
                ]
                w2_sb[s] = [
                    wpool.tile([P, D_MODEL], bf16, name=f"w2_sb{s}{k}", tag=f"w2{s}{k}")
                    for k in range(K2)
                ]

            # No PE warmup filler: full-width filler matmuls read SBUF at
            # ~300GB/s and slow the critical first input DMAs (measured
            # neutral), and 32x32-masked filler does not register as PE-busy
            # for the HAM clock gate (measured: data matmuls stayed cold).
            # The ~1.7us cold-clock cost on the first ~8 data matmuls is
            # unavoidable.

            # ---- bulk inputs on qSP in exact PE consumption order.
            # (qACT is unusable for bulk inputs: dma_start issue on the ACT
            # engine stream blocks the PSUM-drain epilogue ACTs behind it;
            # splitting inputs across queues scrambles arrival order and
            # stalls the PE.)
            def emit_xg(s, t, eng=None):
                eng = eng or nc.sync
                off, Nt = sec_bounds[s][t]
                for k in range(K1):
                    eng.dma_start(
                        out=xg_sb[s][k][:, off:off + Nt],
                        in_=xT[s][k * P:(k + 1) * P, off:off + Nt],
                    )

            def emit_w1_chunk(s, c0, c1):
                for k in range(K1):
                    nc.sync.dma_start(
                        out=w1_sb[s][k][:, c0:c1],
                        in_=w1[s][k * P:(k + 1) * P, c0:c1],
                    )

            def emit_input_dmas(s, first=False, tiles=None):
                if tiles is None:
                    tiles = list(range(len(sec_bounds[s])))
                if first:
                    # head supply: each DMA ring pays its own ~3us cold
                    # latency ramp, so the first matmul group's deps (xg-t0
                    # and w1-c0, consumed k-interleaved) all ride the FRONT
                    # of the same qSP ring, alternating per k — the k=0 pair
                    # completes ~10.3us and the PE stream-follows at cold
                    # rate.  b1a is qACT's first transfer (lands ~10us,
                    # needed at the first activation ~13.8us).
                    nc.scalar.dma_start(out=b1_sb[s][:], in_=b1t[s][:])
                    off, Nt = sec_bounds[s][tiles[0]]
                    for k in range(K1):
                        nc.sync.dma_start(
                            out=xg_sb[s][k][:, off:off + Nt],
                            in_=xT[s][k * P:(k + 1) * P, off:off + Nt],
                        )
                        nc.sync.dma_start(
                            out=w1_sb[s][k][:, 0:WCHUNK],
                            in_=w1[s][k * P:(k + 1) * P, 0:WCHUNK],
                        )
                    for t in tiles[1:]:
                        emit_xg(s, t)
                    c0 = WCHUNK
                else:
                    nc.sync.dma_start(out=b1_sb[s][:], in_=b1t[s][:])
                    for t in tiles:
                        emit_xg(s, t)
                    c0 = 0
                step = WCHUNK if first else 2 * WCHUNK
                bounds = list(range(c0, 2 * FH, step)) + [2 * FH]
                for b0, b1 in zip(bounds[:-1], bounds[1:]):
                    emit_w1_chunk(s, b0, b1)
                for k in range(K2):
                    nc.sync.dma_start(
                        out=w2_sb[s][k][:], in_=w2[s][k * P:(k + 1) * P, :],
                    )

            n_ba = len(sec_bounds["a"])
            a_main = list(range(n_ba - 1)) if has_tail else list(range(n_ba))
            a_tail = [n_ba - 1] if has_tail else []

            emit_input_dmas("a", first=True, tiles=a_main)
            emit_input_dmas("b")
            for t in a_tail:
                emit_xg("a", t)

            # ---- main loops ----
            # W1 columns host-permuted to [a_0 | g_0 | a_1 | g_1 | ...] so the
            # PE reads w1_sb left-to-right.  matmul1 iterates i-outer/t-inner:
            # all token tiles consume one weight block before moving on, so
            # the weight-DMA demand rate is ~halved and tokens (cheap, small)
            # are needed upfront instead of mid-stream.
            uT = {}

            def mm1_group(s, i, t, ps_a, ps_g, ka, kb):
                off, Nt = sec_bounds[s][t]
                for k in range(ka, kb):
                    nc.tensor.matmul(
                        ps_a[:, :Nt],
                        w1_sb[s][k][:, ts(2 * i, P)],
                        xg_sb[s][k][:, off:off + Nt],
                        start=(k == 0),
                        stop=(k == K1 - 1),
                    )
                for k in range(ka, kb):
                    nc.tensor.matmul(
                        ps_g[:, :Nt],
                        w1_sb[s][k][:, ts(2 * i + 1, P)],
                        xg_sb[s][k][:, off:off + Nt],
                        start=(k == 0),
                        stop=(k == K1 - 1),
                    )

            def mm1_epilogue(s, i, t, ps_a, ps_g):
                off, Nt = sec_bounds[s][t]
                a_t = epool.tile([P, NT], bf16, name=f"a{s}{t}_{i}", tag="a")
                nc.scalar.activation(
                    a_t[:, :Nt], ps_a[:, :Nt], AF.Identity,
                    bias=b1_sb[s][:, 2 * i:2 * i + 1],
                )
                g_t = epool.tile([P, NT], bf16, name=f"g{s}{t}_{i}", tag="g")
                if use_silu:
                    nc.scalar.activation(
                        g_t[:, :Nt], ps_g[:, :Nt], AF.Silu,
                        bias=b1_sb[s][:, 2 * i + 1:2 * i + 2],
                    )
                else:
                    s_t = epool.tile(
                        [P, NT], bf16, name=f"s{s}{t}_{i}", tag="s")
                    nc.scalar.activation(
                        s_t[:, :Nt], ps_g[:, :Nt], AF.Sigmoid,
                        bias=b1_sb[s][:, 2 * i + 1:2 * i + 2],
                    )
                    gb_t = epool.tile(
                        [P, NT], bf16, name=f"gb{s}{t}_{i}", tag="gb")
                    nc.scalar.activation(
                        gb_t[:, :Nt], ps_g[:, :Nt], AF.Identity,
                        bias=b1_sb[s][:, 2 * i + 1:2 * i + 2],
                    )
                    nc.vector.tensor_mul(
                        g_t[:, :Nt], gb_t[:, :Nt], s_t[:, :Nt])
                nc.vector.tensor_mul(
                    uT[(s, t)][:, i, :Nt], a_t[:, :Nt], g_t[:, :Nt])

            def mm1_ps(s, i, t):
                ps_a = ps_pool.tile([P, NT], f32, name=f"psa{s}{t}_{i}", tag="ps")
                ps_g = ps_pool.tile([P, NT], f32, name=f"psg{s}{t}_{i}", tag="ps")
                return ps_a, ps_g

            def emit_mm1(s, staggered=False, tiles=None):
                if tiles is None:
                    tiles = list(range(len(sec_bounds[s])))
                for t in tiles:
                    uT[(s, t)] = upool.tile(
                        [P, K2, NT], bf16, name=f"uT{s}{t}", tag="uT", bufs=4)
                if staggered:
                    # first two i-blocks run tile 0 first so compute can
                    # start on just xa_t0 + w1 chunk 0 while the rest streams
                    # in.  (A split-k head start was tried and measured
                    # neutral: the DMA rings complete the whole early batch
                    # together, so finer deps don't move the first matmul.)
                    t0 = tiles[0]
                    sched = [(0, t0), (1, t0)]
                    sched += [(i, t) for i in (0, 1) for t in tiles[1:]]
                    sched += [(i, t) for i in range(2, K2) for t in tiles]
                else:
                    sched = [(i, t) for i in range(K2) for t in tiles]
                for i, t in sched:
                    ps_a, ps_g = mm1_ps(s, i, t)
                    mm1_group(s, i, t, ps_a, ps_g, 0, K1)
                    mm1_epilogue(s, i, t, ps_a, ps_g)

            def emit_mm2(s, tiles=None, packed=False):
                bounds = sec_bounds[s]
                if tiles is None:
                    tiles = list(range(len(bounds)))
                for t in tiles:
                    off, Nt = bounds[t]
                    y_w = None
                    if packed:
                        y_w = epool.tile([P, NO, Nt], bf16, name=f"yw{s}{t}",
                                         tag="yw")
                    for m in range(NO):
                        ps_y = ps_pool.tile(
                            [P, NT], f32, name=f"psy{s}{t}_{m}", tag="ps")
                        for k in range(K2):
                            nc.tensor.matmul(
                                ps_y[:, :Nt],
                                w2_sb[s][k][:, ts(m, P)],
                                uT[(s, t)][:, k, :Nt],
                                start=(k == 0),
                                stop=(k == K2 - 1),
                            )
                        # psum drain on DVE (idle), output via qSP behind the
                        # inputs: keeps the ScalarE stream free for the a/g
                        # drains (dma_start issue on ACT blocks them) and
                        # avoids SWDGE SBUF-read contention with the PE.
                        if packed:
                            nc.vector.tensor_copy(y_w[:, m, :], ps_y[:, :Nt])
                            if m == HNO - 1:
                                # qACT: idle at the tail, while qSP is still
                                # draining y-b issues ~1us behind the copies
                                nc.scalar.dma_start(
                                    out=out_tail_lo, in_=y_w[:, :HNO, :])
                        else:
                            y_t = epool.tile([P, NT], bf16, name=f"y{s}{t}_{m}",
                                             tag="y", bufs=12)
                            nc.vector.tensor_copy(y_t[:, :Nt], ps_y[:, :Nt])
                            o0 = out_off[s] + off
                            nc.sync.dma_start(
                                out=out_all[m * P:(m + 1) * P, o0:o0 + Nt],
                                in_=y_t[:, :Nt],
                            )
                    if packed:
                        nc.scalar.dma_start(out=out_tail_hi, in_=y_w[:, HNO:, :])

            emit_mm1("a", staggered=True, tiles=a_main)
            emit_mm2("a", tiles=a_main)
            emit_mm1("b")
            emit_mm2("b")
            if a_tail:
                emit_mm1("a", tiles=a_tail)
                emit_mm2("a", tiles=a_tail, packed=True)

    nc.compile()
    return nc


def _route_tokens(xf, Wr, temp):
    """Bit-match the reference's router on CPU jax: logits, top-2, softmax."""
    import jax
    import jax.numpy as jnp

    cpu = jax.devices("cpu")[0]
    with jax.default_device(cpu):
        xj = jnp.asarray(xf)
        logits = (xj @ jnp.asarray(Wr)) / jnp.asarray(temp)
        topw, topi = jax.lax.top_k(logits, TOP_K)
        topw = jax.nn.softmax(topw, axis=-1)
    return np.asarray(topi), np.asarray(topw)


def _pad(n):
    # pad only to 4 columns (8-byte bf16 DMA lines); coarser padding streams
    # pure-zero columns through every matmul at ~80ns/column
    return max(P, ((n + 3) // 4) * 4)


def kernel(**inputs) -> np.ndarray:
    global LAST_RESULTS
    from concourse.bass_utils import run_bass_kernel_spmd

    x = np.asarray(inputs["x"], dtype=np.float32)
    Wr = np.asarray(inputs["Wr"], dtype=np.float32)
    temp = np.asarray(inputs["temp"], dtype=np.float32)
    W1 = np.asarray(inputs["W1"], dtype=np.float32)
    b1 = np.asarray(inputs["b1"], dtype=np.float32)
    W2 = np.asarray(inputs["W2"], dtype=np.float32)
    b2 = np.asarray(inputs["b2"], dtype=np.float32)

    B, S, D = x.shape
    T = B * S
    xf = x.reshape(T, D)

    topi, topw = _route_tokens(xf, Wr, temp)

    # Per-expert token lists and combine weights.
    tok_idx = []
    tok_w = []
    for e in range(NUM_EXPERTS):
        mask = topi == e                       # [T, K]
        sel = mask.any(axis=1)
        idx = np.nonzero(sel)[0]
        w = (topw * mask).sum(axis=1)[idx]
        tok_idx.append(idx)
        tok_w.append(w.astype(np.float32))

    counts = np.array([len(i) for i in tok_idx])
    order = np.argsort(-counts, kind="stable")
    bigs = list(order[:4])                     # section A experts
    smalls = list(order[4:])                   # section B experts
    CA = _pad(max(counts[e] for e in bigs))
    CB = _pad(max(counts[e] for e in smalls))

    # a/g interleave within a ff half: [a_0 | g_0 | a_1 | g_1 | ...]
    def w1_cols(h):
        cols = []
        for j in range(h * (FH // P), (h + 1) * (FH // P)):
            cols.append(np.arange(j * P, (j + 1) * P))            # a_j
            cols.append(np.arange(D_FF + j * P, D_FF + (j + 1) * P))  # g_j
        return np.concatenate(cols)

    cols_h = [w1_cols(0), w1_cols(1)]

    bf16 = ml_dtypes.bfloat16

    def xgT_of(e, C):
        idx = tok_idx[e]
        xg = np.zeros((C, D), dtype=np.float32)
        xg[: len(idx)] = xf[idx]
        return np.ascontiguousarray(xg.T).astype(bf16)

    xgT_cache = {e: xgT_of(e, CA if e in bigs else CB) for e in range(NUM_EXPERTS)}

    in_maps = []
    for c in range(N_CORES):
        h = c // 4
        m = {}
        for s, elist in (("a", bigs), ("b", smalls)):
            e = elist[c % 4]
            cols = cols_h[h]
            m[f"x{s}T"] = xgT_cache[e]
            m[f"w1{s}"] = np.ascontiguousarray(W1[e][:, cols]).astype(bf16)
            m[f"w2{s}"] = np.ascontiguousarray(
                W2[e][h * FH:(h + 1) * FH, :]).astype(bf16)
            m[f"b1t{s}"] = np.ascontiguousarray(
                b1[e][cols].reshape(2 * FH // P, P).T)
        in_maps.append(m)

    key = (CA, CB)
    if key not in _NC_CACHE:
        _NC_CACHE[key] = _build_nc(CA, CB)
    nc = _NC_CACHE[key]

    trace = bool(os.environ.get("MOE_KERNEL_TRACE"))
    kwargs = {}
    if trace:
        kwargs = dict(trace=True, trace_cores=list(range(N_CORES)))
    res = run_bass_kernel_spmd(nc, in_maps, core_ids=list(range(N_CORES)), **kwargs)
    LAST_RESULTS = res

    has_tail, CA_main, a_last = _tail_split(CA)

    def sec_out(core, s):
        oa = np.asarray(res.results[core]["outall"]).astype(np.float32)
        if s == "b":
            return oa[:, CA_main:CA_main + CB]
        y = oa[:, :CA_main]
        if has_tail:
            yt = oa[:P, CA_main + CB:]
            yt = yt.reshape(P, NO, a_last).transpose(1, 0, 2).reshape(
                D_MODEL, a_last)
            y = np.concatenate([y, yt], axis=1)
        return y

    out = np.zeros((T, D), dtype=np.float32)
    for s, elist in (("a", bigs), ("b", smalls)):
        for i, e in enumerate(elist):
            idx = tok_idx[e]
            if len(idx) == 0:
                continue
            y0 = sec_out(i, s)
            y1 = sec_out(i + 4, s)
            y = (y0 + y1)[:, : len(idx)].T + b2[e]
            out[idx] += y * tok_w[e][:, None]

    return out.reshape(B, S, D)

